# revision 24
# baseline (speedup 1.0000x reference)
"""GPT-2 style attention block on 8 TRN2 NeuronCores.

Sharding: core c = 2*b + g handles batch b (of 4) and head-group g (of 2,
8 heads each).  Per core everything is computed in a transposed layout
(scores [k, q], attention-out [d, s], proj-out [o, s]) so no on-device
transposes are needed:

  qT/kT  [c, s] = w_chunk.T @ xT            (lhsT = w chunk, rhs = xT)
  v      [s, c] = xT_chunk.T @ w_v          (lhsT = xT chunk, rhs = w_v)
  sT     [k, q] = kT_tile.T @ qT            (lhsT = kT 64x128, rhs = qT)
  eT     = exp(0.125 * sT)   (causal: invalid k>q tiles never computed,
                              diagonal 128x128 blocks masked post-exp)
  aT     [65, q] = [v_h | 1].T @ eT         (row 64 = softmax denominator)
  aT_n   = aT[0:64] * (1/denom)  broadcast  (gpsimd partition_broadcast)
  outT   [o, s] = wp_chunk.T @ aT_n         (partial over local heads)

Host gathers: out[b] = (outT_core(2b) + outT_core(2b+1)).T, k/v shards are
disjoint per core.  Compute dtype bf16 (inputs pre-cast on host), psum f32.
"""

import numpy as np
import ml_dtypes

B, S, E = 4, 2048, 1024
H, D = 16, 64
G = 2            # head groups (tensor-parallel)
HL = H // G      # 8 local heads
CL = HL * D      # 512 local qkv channels
ST = 128         # seq tile (partition dim)
SC = 512         # seq chunk (matmul moving dim)
NST = S // ST    # 16
NSC = S // SC    # 4
EC = E // 128    # 8 embedding chunks
NCORES = 8

BF16 = ml_dtypes.bfloat16

_cached = {}


def build_nc():
    import concourse.bass as bass
    import concourse.tile as tile
    from concourse import bacc, mybir

    f32 = mybir.dt.float32
    bf16 = mybir.dt.bfloat16
    AF = mybir.ActivationFunctionType

    nc = bacc.Bacc(None, target_bir_lowering=False)

    xT = nc.declare_dram_parameter("xT", [E, S], bf16, isOutput=False)
    wqkv = nc.declare_dram_parameter("wqkv", [E, 3 * CL], bf16, isOutput=False)
    wp = nc.declare_dram_parameter("wp", [CL, E], bf16, isOutput=False)
    maskT = nc.declare_dram_parameter("maskT", [ST, ST], bf16, isOutput=False)
    bqk = nc.declare_dram_parameter("bqk", [128, 8], f32, isOutput=False)
    bv = nc.declare_dram_parameter("bv", [128, CL], f32, isOutput=False)
    bproj = nc.declare_dram_parameter("bproj", [128, 8], f32, isOutput=False)
    kt_out = nc.declare_dram_parameter("kt_out", [CL, S], f32, isOutput=True)
    v_out = nc.declare_dram_parameter("v_out", [S, CL], f32, isOutput=True)
    out_t = nc.declare_dram_parameter("out_t", [E, S], f32, isOutput=True)

    with tile.TileContext(nc) as tc:
        with (
            tc.tile_pool(name="persist", bufs=1) as persist,
            tc.tile_pool(name="io", bufs=3) as io,
            tc.tile_pool(name="etp", bufs=5) as etp,
            tc.tile_pool(name="rp", bufs=2) as rp,
            tc.tile_pool(name="outp", bufs=3) as outp,
                    ):
            # ---- persistent SBUF tiles ----
            # interleave x / w loads so the first QKV matmuls start early
            xt = []
            wqt = []
            qs = [nc.sync, nc.scalar, nc.gpsimd]
            for e in range(EC):
                t = persist.tile([128, 3 * CL], bf16, tag=f"wq{e}", name=f"wq{e}")
                qs[e % 3].dma_start(t[:], wqkv[128 * e:128 * (e + 1), :])
                wqt.append(t)
                t2 = persist.tile([128, S], bf16, tag=f"xt{e}", name=f"xt{e}")
                qs[(e + 1) % 3].dma_start(t2[:], xT[128 * e:128 * (e + 1), :])
                xt.append(t2)
            wpt = []
            for cc in range(CL // 128):
                t = persist.tile([128, E], bf16, tag=f"wp{cc}", name=f"wp{cc}")
                nc.scalar.dma_start(t[:], wp[128 * cc:128 * (cc + 1), :])
                wpt.append(t)
            maskt = persist.tile([ST, ST], bf16, tag="maskt", name="maskt")
            nc.scalar.dma_start(maskt[:], maskT[:])
            bqkt = persist.tile([128, 8], f32, tag="bqkt", name="bqkt")
            nc.scalar.dma_start(bqkt[:], bqk[:])
            bvt = persist.tile([128, CL], f32, tag="bvt", name="bvt")
            nc.scalar.dma_start(bvt[:], bv[:])
            bprojt = persist.tile([128, 8], f32, tag="bprojt", name="bprojt")
            nc.scalar.dma_start(bprojt[:], bproj[:])

            # persistent compute tensors.  qtd/ktd hold each head's 64
            # channels DUPLICATED on partitions 0-63 and 64-127: the score
            # matmuls then contract over K=128 (scores doubled; exp scale
            # halved), which keeps the PE activity monitor at full clock.
            qtd = [persist.tile([128, S], bf16, tag=f"qtd{h}", name=f"qtd{h}")
                   for h in range(HL)]
            ktd = [persist.tile([128, S], bf16, tag=f"ktd{h}", name=f"ktd{h}")
                   for h in range(HL)]
            vaug = [persist.tile([128, HL * (D + 1)], bf16, tag=f"va{j}",
                                 name=f"va{j}") for j in range(NST)]
            atb = [persist.tile([128, S], bf16, tag=f"atb{i}", name=f"atb{i}")
                   for i in range(CL // 128)]

            # ones columns of v_aug (col 64 of each 65-block stays 1.0)
            for j in range(NST):
                nc.vector.memset(vaug[j][:], 1.0)

            # ---- phase 1: QKV projections ----
            # single persistent psum pools (shared with attention/proj);
            # emission interleaved so attention can start on early heads
            sps = tc.alloc_tile_pool(name="sps", bufs=4, space="PSUM")
            aps = tc.alloc_tile_pool(name="aps", bufs=4, space="PSUM")

            def qk_chunk(cc):
                ps = [sps.tile([128, SC], f32, tag="sps", name=f"qkp{cc}_{sc}")
                      for sc in range(NSC)]
                for e in range(EC):
                    for sc in range(NSC):
                        nc.tensor.matmul(
                            ps[sc][:],
                            wqt[e][:, 128 * cc:128 * (cc + 1)],
                            xt[e][:, SC * sc:SC * (sc + 1)],
                            start=(e == 0), stop=(e == EC - 1),
                        )
                for sc in range(NSC):
                    ss = slice(SC * sc, SC * (sc + 1))
                    if cc < 4:  # q -> duplicated bf16
                        dst = qtd
                    else:
                        kk = cc - 4
                        ktf = io.tile([128, SC], f32, tag="ktf",
                                      name=f"ktf{cc}_{sc}")
                        nc.scalar.activation(ktf[:], ps[sc][:], AF.Identity,
                                             bias=bqkt[:, cc:cc + 1])
                        nc.sync.dma_start(
                            kt_out[128 * kk:128 * (kk + 1), ss], ktf[:])
                        dst = ktd
                    hc = 2 * (cc % 4)  # first head in this 128-chunk
                    for half in range(2):
                        hp = slice(64 * half, 64 * half + 64)
                        bias = bqkt[hp, cc:cc + 1]
                        nc.vector.tensor_scalar_add(
                            dst[hc + half][0:64, ss], ps[sc][hp, :], bias)
                        nc.scalar.activation(
                            dst[hc + half][64:128, ss], ps[sc][hp, :],
                            AF.Identity, bias=bias)

            def v_tiles(sts):
                for st in sts:
                    vp = aps.tile([128, CL], f32, tag="aps", name=f"vp{st}")
                    for e in range(EC):
                        nc.tensor.matmul(
                            vp[:],
                            xt[e][:, ST * st:ST * (st + 1)],
                            wqt[e][:, 2 * CL:3 * CL],
                            start=(e == 0), stop=(e == EC - 1),
                        )
                    vf = io.tile([128, CL], f32, tag="vf", name=f"vf{st}")
                    nc.vector.tensor_add(vf[:], vp[:], bvt[:])
                    nc.gpsimd.dma_start(v_out[ST * st:ST * (st + 1), :], vf[:])
                    nc.vector.tensor_add(
                        vaug[st][:].rearrange("p (h x) -> p h x", h=HL)[:, :, 0:D],
                        vp[:].rearrange("p (h d) -> p h d", h=HL),
                        bvt[:].rearrange("p (h d) -> p h d", h=HL),
                    )

            qk_chunk(0)
            qk_chunk(4)
            v_tiles(range(0, 8))
            qk_chunk(1)
            qk_chunk(5)
            v_tiles(range(8, 16))
            qk_chunk(2)
            qk_chunk(6)
            qk_chunk(3)
            qk_chunk(7)

            # ---- phase 2+3: attention (two heads interleaved, pair-major)
            # with the output projection overlapped per finished q-range ----
            if True:
                for p in range(2):  # q-chunk pairs (0,1) and (2,3)
                    qcs = [2 * p, 2 * p + 1]
                    for hh in range(HL // 2):  # head pairs, interleaved
                        heads = [2 * hh, 2 * hh + 1]
                        apsum = {}
                        for h in heads:
                            for qc in qcs:
                                apsum[h, qc] = aps.tile(
                                    [D + 1, SC], f32, tag="aps",
                                    name=f"ap{h}_{qc}")
                        for j in range(8 * p + 8):
                            for h in heads:
                                kT_h = ktd[h]
                                qT_h = qtd[h]
                                for qc in qcs:
                                    if j > 4 * qc + 3:
                                        continue
                                    q_lo = max(SC * qc, ST * j)
                                    n = SC * (qc + 1) - q_lo
                                    sp = sps.tile([128, SC], f32, tag="sps",
                                                  name=f"sp{h}_{j}_{qc}")
                                    nc.tensor.matmul(
                                        sp[:, 0:n],
                                        kT_h[:][:, ST * j:ST * (j + 1)],
                                        qT_h[:][:, q_lo:q_lo + n],
                                        start=True, stop=True)
                                    et = etp.tile([128, SC], bf16, tag="et",
                                                  name=f"et{h}_{j}_{qc}")
                                    nc.scalar.activation(et[:, 0:n], sp[:, 0:n],
                                                         AF.Exp, scale=0.0625)
                                    if q_lo == ST * j:  # diagonal block
                                        nc.vector.tensor_mul(
                                            et[:, 0:ST], et[:, 0:ST], maskt[:])
                                    nc.tensor.matmul(
                                        apsum[h, qc][:, q_lo - SC * qc:
                                                      q_lo - SC * qc + n],
                                        vaug[j][:, (D + 1) * h:(D + 1) * (h + 1)],
                                        et[:, 0:n],
                                        start=(j == 0), stop=(j == 4 * qc + 3),
                                    )
                        for h in heads:
                            ki = h // 2
                            pr = 64 * (h % 2)
                            for qc in qcs:
                                dc = rp.tile([1, SC], f32, tag="dc",
                                             name=f"dc{h}_{qc}")
                                nc.vector.tensor_copy(dc[:],
                                                      apsum[h, qc][D:D + 1, :])
                                dn = rp.tile([1, SC], f32, tag="dn",
                                             name=f"dn{h}_{qc}")
                                nc.vector.reciprocal_approx_fast(
                                    out=dn[:], in_=dc[:])
                                rb = rp.tile([64, SC], f32, tag="rb",
                                             name=f"rb{h}_{qc}")
                                nc.gpsimd.partition_broadcast(rb[:], dn[:])
                                nc.vector.tensor_mul(
                                    atb[ki][pr:pr + 64, SC * qc:SC * (qc + 1)],
                                    apsum[h, qc][0:D, :], rb[:])
                    # output projection for this pair's q-range (reuses
                    # the score-psum slots; overlaps next pair's attention)
                    for sc in qcs:
                        for oc in range(E // 128):
                            pp = sps.tile([128, SC], f32, tag="sps",
                                          name=f"pp{oc}_{sc}")
                            for cc in range(CL // 128):
                                nc.tensor.matmul(
                                    pp[:],
                                    wpt[cc][:, 128 * oc:128 * (oc + 1)],
                                    atb[cc][:, SC * sc:SC * (sc + 1)],
                                    start=(cc == 0), stop=(cc == CL // 128 - 1),
                                )
                            ot = outp.tile([128, SC], f32, tag="ot",
                                           name=f"ot{oc}_{sc}")
                            nc.vector.tensor_scalar_add(ot[:], pp[:],
                                                        bprojt[:, oc:oc + 1])
                            nc.sync.dma_start(
                                out_t[128 * oc:128 * (oc + 1),
                                      SC * sc:SC * (sc + 1)],
                                ot[:])
            aps.release()
            sps.release()
    nc.finalize()
    return nc


def make_in_maps(hidden_states, w_attn, b_attn, w_proj, b_proj):
    x = np.asarray(hidden_states, dtype=np.float32)
    wa = np.asarray(w_attn, dtype=np.float32)
    ba = np.asarray(b_attn, dtype=np.float32)
    wpj = np.asarray(w_proj, dtype=np.float32)
    bp = np.asarray(b_proj, dtype=np.float32)

    kk, qq = np.meshgrid(np.arange(ST), np.arange(ST), indexing="ij")
    maskT = (qq >= kk).astype(BF16)

    in_maps = []
    for c in range(NCORES):
        b, g = c // G, c % G
        cs = slice(CL * g, CL * (g + 1))
        wq = np.concatenate(
            [wa[:, CL * g:CL * (g + 1)],
             wa[:, E + CL * g:E + CL * (g + 1)],
             wa[:, 2 * E + CL * g:2 * E + CL * (g + 1)]], axis=1)
        bqk = np.zeros((128, 8), np.float32)
        for cc in range(4):
            bqk[:, cc] = ba[CL * g + 128 * cc:CL * g + 128 * (cc + 1)]
            bqk[:, cc + 4] = ba[E + CL * g + 128 * cc:E + CL * g + 128 * (cc + 1)]
        bv = np.tile(ba[2 * E + CL * g:2 * E + CL * (g + 1)][None, :], (128, 1))
        bproj = np.zeros((128, 8), np.float32)
        if g == 0:
            for oc in range(8):
                bproj[:, oc] = bp[128 * oc:128 * (oc + 1)]
        in_maps.append({
            "xT": np.ascontiguousarray(x[b].T).astype(BF16),
            "wqkv": np.ascontiguousarray(wq).astype(BF16),
            "wp": np.ascontiguousarray(wpj[cs, :]).astype(BF16),
            "maskT": maskT,
            "bqk": bqk.astype(np.float32),
            "bv": bv.astype(np.float32),
            "bproj": bproj.astype(np.float32),
        })
    return in_maps


def gather(results):
    out = np.empty((B, S, E), np.float32)
    k_ret = np.empty((B, H, D, S), np.float32)
    v_ret = np.empty((B, H, S, D), np.float32)
    for b in range(B):
        p0 = np.asarray(results[G * b]["out_t"], np.float32)
        p1 = np.asarray(results[G * b + 1]["out_t"], np.float32)
        out[b] = (p0 + p1).T
        for g in range(G):
            r = results[G * b + g]
            k_ret[b, HL * g:HL * (g + 1)] = np.asarray(
                r["kt_out"], np.float32).reshape(HL, D, S)
            v_ret[b, HL * g:HL * (g + 1)] = np.asarray(
                r["v_out"], np.float32).reshape(S, HL, D).transpose(1, 0, 2)
    return out, k_ret, v_ret


def run_on_hw(in_maps, trace=False, **kw):
    from concourse.bass_utils import run_bass_kernel_spmd
    if "nc" not in _cached:
        _cached["nc"] = build_nc()
    return run_bass_kernel_spmd(_cached["nc"], in_maps,
                                core_ids=list(range(NCORES)), trace=trace, **kw)


def kernel(hidden_states, w_attn, b_attn, w_proj, b_proj):
    in_maps = make_in_maps(hidden_states, w_attn, b_attn, w_proj, b_proj)
    res = run_on_hw(in_maps)
    return gather(res.results)


# revision 25
# speedup vs baseline: 1.0124x; 1.0124x over previous
"""GPT-2 style attention block on 8 TRN2 NeuronCores.

Sharding: core c = 2*b + g handles batch b (of 4) and head-group g (of 2,
8 heads each).  Per core everything is computed in a transposed layout
(scores [k, q], attention-out [d, s], proj-out [o, s]) so no on-device
transposes are needed:

  qT/kT  [c, s] = w_chunk.T @ xT            (lhsT = w chunk, rhs = xT)
  v      [s, c] = xT_chunk.T @ w_v          (lhsT = xT chunk, rhs = w_v)
  sT     [k, q] = kT_tile.T @ qT            (lhsT = kT 64x128, rhs = qT)
  eT     = exp(0.125 * sT)   (causal: invalid k>q tiles never computed,
                              diagonal 128x128 blocks masked post-exp)
  aT     [65, q] = [v_h | 1].T @ eT         (row 64 = softmax denominator)
  aT_n   = aT[0:64] * (1/denom)  broadcast  (gpsimd partition_broadcast)
  outT   [o, s] = wp_chunk.T @ aT_n         (partial over local heads)

Host gathers: out[b] = (outT_core(2b) + outT_core(2b+1)).T, k/v shards are
disjoint per core.  Compute dtype bf16 (inputs pre-cast on host), psum f32.
"""

import numpy as np
import ml_dtypes

B, S, E = 4, 2048, 1024
H, D = 16, 64
G = 2            # head groups (tensor-parallel)
HL = H // G      # 8 local heads
CL = HL * D      # 512 local qkv channels
ST = 128         # seq tile (partition dim)
SC = 512         # seq chunk (matmul moving dim)
NST = S // ST    # 16
NSC = S // SC    # 4
EC = E // 128    # 8 embedding chunks
NCORES = 8

BF16 = ml_dtypes.bfloat16

_cached = {}


def build_nc():
    import concourse.bass as bass
    import concourse.tile as tile
    from concourse import bacc, mybir

    f32 = mybir.dt.float32
    bf16 = mybir.dt.bfloat16
    AF = mybir.ActivationFunctionType

    nc = bacc.Bacc(None, target_bir_lowering=False)

    xT = nc.declare_dram_parameter("xT", [E, S], bf16, isOutput=False)
    wqkv = nc.declare_dram_parameter("wqkv", [E, 3 * CL], bf16, isOutput=False)
    wp = nc.declare_dram_parameter("wp", [CL, E], bf16, isOutput=False)
    maskT = nc.declare_dram_parameter("maskT", [ST, ST], bf16, isOutput=False)
    bqk = nc.declare_dram_parameter("bqk", [128, 8], f32, isOutput=False)
    bv = nc.declare_dram_parameter("bv", [128, CL], f32, isOutput=False)
    bproj = nc.declare_dram_parameter("bproj", [128, 8], f32, isOutput=False)
    kt_out = nc.declare_dram_parameter("kt_out", [CL, S], f32, isOutput=True)
    v_out = nc.declare_dram_parameter("v_out", [S, CL], f32, isOutput=True)
    out_t = nc.declare_dram_parameter("out_t", [E, S], f32, isOutput=True)

    with tile.TileContext(nc) as tc:
        with (
            tc.tile_pool(name="persist", bufs=1) as persist,
            tc.tile_pool(name="io", bufs=3) as io,
            tc.tile_pool(name="etp", bufs=5) as etp,
            tc.tile_pool(name="rp", bufs=2) as rp,
            tc.tile_pool(name="outp", bufs=3) as outp,
                    ):
            # ---- persistent SBUF tiles ----
            # interleave x / w loads so the first QKV matmuls start early
            xt = []
            wqt = []
            for e in range(EC):
                t = persist.tile([128, 3 * CL], bf16, tag=f"wq{e}", name=f"wq{e}")
                nc.gpsimd.dma_start(t[:], wqkv[128 * e:128 * (e + 1), :])
                wqt.append(t)
                t2 = persist.tile([128, S], bf16, tag=f"xt{e}", name=f"xt{e}")
                (nc.sync if e % 2 == 0 else nc.scalar).dma_start(
                    t2[:], xT[128 * e:128 * (e + 1), :])
                xt.append(t2)
            wpt = []
            for cc in range(CL // 128):
                t = persist.tile([128, E], bf16, tag=f"wp{cc}", name=f"wp{cc}")
                nc.scalar.dma_start(t[:], wp[128 * cc:128 * (cc + 1), :])
                wpt.append(t)
            maskt = persist.tile([ST, ST], bf16, tag="maskt", name="maskt")
            nc.scalar.dma_start(maskt[:], maskT[:])
            bqkt = persist.tile([128, 8], f32, tag="bqkt", name="bqkt")
            nc.scalar.dma_start(bqkt[:], bqk[:])
            bvt = persist.tile([128, CL], f32, tag="bvt", name="bvt")
            nc.scalar.dma_start(bvt[:], bv[:])
            bprojt = persist.tile([128, 8], f32, tag="bprojt", name="bprojt")
            nc.scalar.dma_start(bprojt[:], bproj[:])

            # persistent compute tensors.  qtd/ktd hold each head's 64
            # channels DUPLICATED on partitions 0-63 and 64-127: the score
            # matmuls then contract over K=128 (scores doubled; exp scale
            # halved), which keeps the PE activity monitor at full clock.
            qtd = [persist.tile([128, S], bf16, tag=f"qtd{h}", name=f"qtd{h}")
                   for h in range(HL)]
            ktd = [persist.tile([128, S], bf16, tag=f"ktd{h}", name=f"ktd{h}")
                   for h in range(HL)]
            vaug = [persist.tile([128, HL * (D + 1)], bf16, tag=f"va{j}",
                                 name=f"va{j}") for j in range(NST)]
            atb = [persist.tile([128, S], bf16, tag=f"atb{i}", name=f"atb{i}")
                   for i in range(CL // 128)]

            # ones columns of v_aug (col 64 of each 65-block stays 1.0)
            for j in range(NST):
                nc.vector.memset(vaug[j][:], 1.0)

            # ---- phase 1: QKV projections ----
            # single persistent psum pools (shared with attention/proj);
            # emission interleaved so attention can start on early heads
            sps = tc.alloc_tile_pool(name="sps", bufs=4, space="PSUM")
            aps = tc.alloc_tile_pool(name="aps", bufs=4, space="PSUM")

            def qk_chunk(cc):
                ps = [sps.tile([128, SC], f32, tag="sps", name=f"qkp{cc}_{sc}")
                      for sc in range(NSC)]
                for e in range(EC):
                    for sc in range(NSC):
                        nc.tensor.matmul(
                            ps[sc][:],
                            wqt[e][:, 128 * cc:128 * (cc + 1)],
                            xt[e][:, SC * sc:SC * (sc + 1)],
                            start=(e == 0), stop=(e == EC - 1),
                        )
                for sc in range(NSC):
                    ss = slice(SC * sc, SC * (sc + 1))
                    if cc < 4:  # q -> duplicated bf16
                        dst = qtd
                    else:
                        kk = cc - 4
                        ktf = io.tile([128, SC], f32, tag="ktf",
                                      name=f"ktf{cc}_{sc}")
                        nc.scalar.activation(ktf[:], ps[sc][:], AF.Identity,
                                             bias=bqkt[:, cc:cc + 1])
                        nc.sync.dma_start(
                            kt_out[128 * kk:128 * (kk + 1), ss], ktf[:])
                        dst = ktd
                    hc = 2 * (cc % 4)  # first head in this 128-chunk
                    for half in range(2):
                        hp = slice(64 * half, 64 * half + 64)
                        bias = bqkt[hp, cc:cc + 1]
                        nc.vector.tensor_scalar_add(
                            dst[hc + half][0:64, ss], ps[sc][hp, :], bias)
                        nc.scalar.activation(
                            dst[hc + half][64:128, ss], ps[sc][hp, :],
                            AF.Identity, bias=bias)

            def v_tiles(sts):
                for st in sts:
                    vp = aps.tile([128, CL], f32, tag="aps", name=f"vp{st}")
                    for e in range(EC):
                        nc.tensor.matmul(
                            vp[:],
                            xt[e][:, ST * st:ST * (st + 1)],
                            wqt[e][:, 2 * CL:3 * CL],
                            start=(e == 0), stop=(e == EC - 1),
                        )
                    vf = io.tile([128, CL], f32, tag="vf", name=f"vf{st}")
                    nc.vector.tensor_add(vf[:], vp[:], bvt[:])
                    nc.gpsimd.dma_start(v_out[ST * st:ST * (st + 1), :], vf[:])
                    nc.vector.tensor_add(
                        vaug[st][:].rearrange("p (h x) -> p h x", h=HL)[:, :, 0:D],
                        vp[:].rearrange("p (h d) -> p h d", h=HL),
                        bvt[:].rearrange("p (h d) -> p h d", h=HL),
                    )

            qk_chunk(0)
            qk_chunk(4)
            v_tiles(range(0, 8))
            qk_chunk(1)
            qk_chunk(5)
            v_tiles(range(8, 16))
            qk_chunk(2)
            qk_chunk(6)
            qk_chunk(3)
            qk_chunk(7)

            # ---- phase 2+3: attention (two heads interleaved, pair-major)
            # with the output projection overlapped per finished q-range ----
            if True:
                for p in range(2):  # q-chunk pairs (0,1) and (2,3)
                    qcs = [2 * p, 2 * p + 1]
                    for hh in range(HL // 2):  # head pairs, interleaved
                        heads = [2 * hh, 2 * hh + 1]
                        apsum = {}
                        for h in heads:
                            for qc in qcs:
                                apsum[h, qc] = aps.tile(
                                    [D + 1, SC], f32, tag="aps",
                                    name=f"ap{h}_{qc}")
                        for j in range(8 * p + 8):
                            for h in heads:
                                kT_h = ktd[h]
                                qT_h = qtd[h]
                                for qc in qcs:
                                    if j > 4 * qc + 3:
                                        continue
                                    q_lo = max(SC * qc, ST * j)
                                    n = SC * (qc + 1) - q_lo
                                    sp = sps.tile([128, SC], f32, tag="sps",
                                                  name=f"sp{h}_{j}_{qc}")
                                    nc.tensor.matmul(
                                        sp[:, 0:n],
                                        kT_h[:][:, ST * j:ST * (j + 1)],
                                        qT_h[:][:, q_lo:q_lo + n],
                                        start=True, stop=True)
                                    et = etp.tile([128, SC], bf16, tag="et",
                                                  name=f"et{h}_{j}_{qc}")
                                    nc.scalar.activation(et[:, 0:n], sp[:, 0:n],
                                                         AF.Exp, scale=0.0625)
                                    if q_lo == ST * j:  # diagonal block
                                        nc.vector.tensor_mul(
                                            et[:, 0:ST], et[:, 0:ST], maskt[:])
                                    nc.tensor.matmul(
                                        apsum[h, qc][:, q_lo - SC * qc:
                                                      q_lo - SC * qc + n],
                                        vaug[j][:, (D + 1) * h:(D + 1) * (h + 1)],
                                        et[:, 0:n],
                                        start=(j == 0), stop=(j == 4 * qc + 3),
                                    )
                        for h in heads:
                            ki = h // 2
                            pr = 64 * (h % 2)
                            for qc in qcs:
                                dc = rp.tile([1, SC], f32, tag="dc",
                                             name=f"dc{h}_{qc}")
                                nc.vector.tensor_copy(dc[:],
                                                      apsum[h, qc][D:D + 1, :])
                                dn = rp.tile([1, SC], f32, tag="dn",
                                             name=f"dn{h}_{qc}")
                                nc.vector.reciprocal_approx_fast(
                                    out=dn[:], in_=dc[:])
                                rb = rp.tile([64, SC], f32, tag="rb",
                                             name=f"rb{h}_{qc}")
                                nc.gpsimd.partition_broadcast(rb[:], dn[:])
                                nc.vector.tensor_mul(
                                    atb[ki][pr:pr + 64, SC * qc:SC * (qc + 1)],
                                    apsum[h, qc][0:D, :], rb[:])
                    # output projection for this pair's q-range (reuses
                    # the score-psum slots; overlaps next pair's attention)
                    for sc in qcs:
                        for oc in range(E // 128):
                            pp = sps.tile([128, SC], f32, tag="sps",
                                          name=f"pp{oc}_{sc}")
                            for cc in range(CL // 128):
                                nc.tensor.matmul(
                                    pp[:],
                                    wpt[cc][:, 128 * oc:128 * (oc + 1)],
                                    atb[cc][:, SC * sc:SC * (sc + 1)],
                                    start=(cc == 0), stop=(cc == CL // 128 - 1),
                                )
                            ot = outp.tile([128, SC], f32, tag="ot",
                                           name=f"ot{oc}_{sc}")
                            nc.vector.tensor_scalar_add(ot[:], pp[:],
                                                        bprojt[:, oc:oc + 1])
                            nc.sync.dma_start(
                                out_t[128 * oc:128 * (oc + 1),
                                      SC * sc:SC * (sc + 1)],
                                ot[:])
            aps.release()
            sps.release()
    nc.finalize()
    return nc


def make_in_maps(hidden_states, w_attn, b_attn, w_proj, b_proj):
    x = np.asarray(hidden_states, dtype=np.float32)
    wa = np.asarray(w_attn, dtype=np.float32)
    ba = np.asarray(b_attn, dtype=np.float32)
    wpj = np.asarray(w_proj, dtype=np.float32)
    bp = np.asarray(b_proj, dtype=np.float32)

    kk, qq = np.meshgrid(np.arange(ST), np.arange(ST), indexing="ij")
    maskT = (qq >= kk).astype(BF16)

    in_maps = []
    for c in range(NCORES):
        b, g = c // G, c % G
        cs = slice(CL * g, CL * (g + 1))
        wq = np.concatenate(
            [wa[:, CL * g:CL * (g + 1)],
             wa[:, E + CL * g:E + CL * (g + 1)],
             wa[:, 2 * E + CL * g:2 * E + CL * (g + 1)]], axis=1)
        bqk = np.zeros((128, 8), np.float32)
        for cc in range(4):
            bqk[:, cc] = ba[CL * g + 128 * cc:CL * g + 128 * (cc + 1)]
            bqk[:, cc + 4] = ba[E + CL * g + 128 * cc:E + CL * g + 128 * (cc + 1)]
        bv = np.tile(ba[2 * E + CL * g:2 * E + CL * (g + 1)][None, :], (128, 1))
        bproj = np.zeros((128, 8), np.float32)
        if g == 0:
            for oc in range(8):
                bproj[:, oc] = bp[128 * oc:128 * (oc + 1)]
        in_maps.append({
            "xT": np.ascontiguousarray(x[b].T).astype(BF16),
            "wqkv": np.ascontiguousarray(wq).astype(BF16),
            "wp": np.ascontiguousarray(wpj[cs, :]).astype(BF16),
            "maskT": maskT,
            "bqk": bqk.astype(np.float32),
            "bv": bv.astype(np.float32),
            "bproj": bproj.astype(np.float32),
        })
    return in_maps


def gather(results):
    out = np.empty((B, S, E), np.float32)
    k_ret = np.empty((B, H, D, S), np.float32)
    v_ret = np.empty((B, H, S, D), np.float32)
    for b in range(B):
        p0 = np.asarray(results[G * b]["out_t"], np.float32)
        p1 = np.asarray(results[G * b + 1]["out_t"], np.float32)
        out[b] = (p0 + p1).T
        for g in range(G):
            r = results[G * b + g]
            k_ret[b, HL * g:HL * (g + 1)] = np.asarray(
                r["kt_out"], np.float32).reshape(HL, D, S)
            v_ret[b, HL * g:HL * (g + 1)] = np.asarray(
                r["v_out"], np.float32).reshape(S, HL, D).transpose(1, 0, 2)
    return out, k_ret, v_ret


def run_on_hw(in_maps, trace=False, **kw):
    from concourse.bass_utils import run_bass_kernel_spmd
    if "nc" not in _cached:
        _cached["nc"] = build_nc()
    return run_bass_kernel_spmd(_cached["nc"], in_maps,
                                core_ids=list(range(NCORES)), trace=trace, **kw)


def kernel(hidden_states, w_attn, b_attn, w_proj, b_proj):
    in_maps = make_in_maps(hidden_states, w_attn, b_attn, w_proj, b_proj)
    res = run_on_hw(in_maps)
    return gather(res.results)


# revision 34
# speedup vs baseline: 1.0578x; 1.0448x over previous
"""GPT-2 style attention block on 8 TRN2 NeuronCores.

Sharding: core c = 2*b + g handles batch b (of 4) and head-group g (of 2,
8 heads each).  Per core everything is computed in a transposed layout
(scores [k, q], attention-out [d, s], proj-out [o, s]) so no on-device
transposes are needed:

  qT/kT  [c, s] = w_chunk.T @ xT            (lhsT = w chunk, rhs = xT)
  v      [s, c] = xT_chunk.T @ w_v          (lhsT = xT chunk, rhs = w_v)
  sT     [k, q] = kT_tile.T @ qT            (lhsT = kT 64x128, rhs = qT)
  eT     = exp(0.125 * sT)   (causal: invalid k>q tiles never computed,
                              diagonal 128x128 blocks masked post-exp)
  aT     [65, q] = [v_h | 1].T @ eT         (row 64 = softmax denominator)
  aT_n   = aT[0:64] * (1/denom)  broadcast  (gpsimd partition_broadcast)
  outT   [o, s] = wp_chunk.T @ aT_n         (partial over local heads)

Host gathers: out[b] = (outT_core(2b) + outT_core(2b+1)).T, k/v shards are
disjoint per core.  Compute dtype bf16 (inputs pre-cast on host), psum f32.
"""

import numpy as np
import ml_dtypes

B, S, E = 4, 2048, 1024
H, D = 16, 64
G = 2            # head groups (tensor-parallel)
HL = H // G      # 8 local heads
CL = HL * D      # 512 local qkv channels
ST = 128         # seq tile (partition dim)
SC = 512         # seq chunk (matmul moving dim)
NST = S // ST    # 16
NSC = S // SC    # 4
EC = E // 128    # 8 embedding chunks
NCORES = 8

BF16 = ml_dtypes.bfloat16

_cached = {}


def build_nc():
    import concourse.bass as bass
    import concourse.tile as tile
    from concourse import bacc, mybir

    f32 = mybir.dt.float32
    bf16 = mybir.dt.bfloat16
    AF = mybir.ActivationFunctionType

    nc = bacc.Bacc(None, target_bir_lowering=False)

    xT = nc.declare_dram_parameter("xT", [E, S], bf16, isOutput=False)
    wqkv = nc.declare_dram_parameter("wqkv", [E, 3 * CL], bf16, isOutput=False)
    wp = nc.declare_dram_parameter("wp", [CL, E], bf16, isOutput=False)
    maskT = nc.declare_dram_parameter("maskT", [ST, ST], bf16, isOutput=False)
    bqk = nc.declare_dram_parameter("bqk", [128, 8], f32, isOutput=False)
    bv = nc.declare_dram_parameter("bv", [128, CL], f32, isOutput=False)
    bproj = nc.declare_dram_parameter("bproj", [128, 8], f32, isOutput=False)
    kt_out = nc.declare_dram_parameter("kt_out", [CL, S], f32, isOutput=True)
    v_out = nc.declare_dram_parameter("v_out", [S, CL], f32, isOutput=True)
    out_t = nc.declare_dram_parameter("out_t", [E, S], f32, isOutput=True)

    with tile.TileContext(nc) as tc:
        with (
            tc.tile_pool(name="persist", bufs=1) as persist,
            tc.tile_pool(name="io", bufs=3) as io,
            tc.tile_pool(name="etp", bufs=5) as etp,
            tc.tile_pool(name="rp", bufs=2) as rp,
            tc.tile_pool(name="outp", bufs=3) as outp,
                    ):
            # ---- persistent SBUF tiles ----
            # interleave x / w loads so the first QKV matmuls start early
            xt = []
            wqt = []
            qs = [nc.gpsimd, nc.sync, nc.scalar]
            for e in range(EC):
                t = persist.tile([128, 3 * CL], bf16, tag=f"wq{e}", name=f"wq{e}")
                qs[e % 3].dma_start(t[:], wqkv[128 * e:128 * (e + 1), :])
                wqt.append(t)
                t2 = persist.tile([128, S], bf16, tag=f"xt{e}", name=f"xt{e}")
                qs[(e + 2) % 3].dma_start(t2[:], xT[128 * e:128 * (e + 1), :])
                xt.append(t2)
            wpt = []
            for cc in range(CL // 128):
                t = persist.tile([128, E], bf16, tag=f"wp{cc}", name=f"wp{cc}")
                nc.scalar.dma_start(t[:], wp[128 * cc:128 * (cc + 1), :])
                wpt.append(t)
            maskt = persist.tile([ST, ST], bf16, tag="maskt", name="maskt")
            nc.scalar.dma_start(maskt[:], maskT[:])
            bqkt = persist.tile([128, 8], f32, tag="bqkt", name="bqkt")
            nc.scalar.dma_start(bqkt[:], bqk[:])
            bvt = persist.tile([128, CL], f32, tag="bvt", name="bvt")
            nc.scalar.dma_start(bvt[:], bv[:])
            bprojt = persist.tile([128, 8], f32, tag="bprojt", name="bprojt")
            nc.scalar.dma_start(bprojt[:], bproj[:])

            # persistent compute tensors.  qtd/ktd hold each head's 64
            # channels DUPLICATED on partitions 0-63 and 64-127: the score
            # matmuls then contract over K=128 (scores doubled; exp scale
            # halved), which keeps the PE activity monitor at full clock.
            qtd = [persist.tile([128, S], bf16, tag=f"qtd{h}", name=f"qtd{h}")
                   for h in range(HL)]
            ktd = [persist.tile([128, S], bf16, tag=f"ktd{h}", name=f"ktd{h}")
                   for h in range(HL)]
            vaug = [persist.tile([128, HL * (D + 1)], bf16, tag=f"va{j}",
                                 name=f"va{j}") for j in range(NST)]
            atb = [persist.tile([128, S], bf16, tag=f"atb{i}", name=f"atb{i}")
                   for i in range(CL // 128)]

            # ones columns of v_aug (col 64 of each 65-block stays 1.0)
            for j in range(NST):
                nc.vector.memset(vaug[j][:], 1.0)

            # ---- phase 1: QKV projections ----
            # single persistent psum pools (shared with attention/proj);
            # emission interleaved so attention can start on early heads
            sps = tc.alloc_tile_pool(name="sps", bufs=3, space="PSUM")
            aps = tc.alloc_tile_pool(name="aps", bufs=5, space="PSUM")

            def qk_chunk(cc):
                ps = [sps.tile([128, SC], f32, tag="sps", name=f"qkp{cc}_{sc}")
                      for sc in range(NSC)]
                for e in range(EC):
                    for sc in range(NSC):
                        nc.tensor.matmul(
                            ps[sc][:],
                            wqt[e][:, 128 * cc:128 * (cc + 1)],
                            xt[e][:, SC * sc:SC * (sc + 1)],
                            start=(e == 0), stop=(e == EC - 1),
                        )
                for sc in range(NSC):
                    ss = slice(SC * sc, SC * (sc + 1))
                    if cc < 4:  # q -> duplicated bf16
                        dst = qtd
                    else:
                        kk = cc - 4
                        ktf = io.tile([128, SC], f32, tag="ktf",
                                      name=f"ktf{cc}_{sc}")
                        nc.scalar.activation(ktf[:], ps[sc][:], AF.Identity,
                                             bias=bqkt[:, cc:cc + 1])
                        nc.sync.dma_start(
                            kt_out[128 * kk:128 * (kk + 1), ss], ktf[:])
                        dst = ktd
                    hc = 2 * (cc % 4)  # first head in this 128-chunk
                    for half in range(2):
                        hp = slice(64 * half, 64 * half + 64)
                        bias = bqkt[hp, cc:cc + 1]
                        nc.vector.tensor_scalar_add(
                            dst[hc + half][0:64, ss], ps[sc][hp, :], bias)
                        nc.sync.dma_start(
                            dst[hc + half][64:128, ss], dst[hc + half][0:64, ss])

            def v_tiles(sts):
                for st in sts:
                    vp = aps.tile([128, CL], f32, tag="aps", name=f"vp{st}")
                    for e in range(EC):
                        nc.tensor.matmul(
                            vp[:],
                            xt[e][:, ST * st:ST * (st + 1)],
                            wqt[e][:, 2 * CL:3 * CL],
                            start=(e == 0), stop=(e == EC - 1),
                        )
                    vf = io.tile([128, CL], f32, tag="vf", name=f"vf{st}")
                    nc.vector.tensor_add(vf[:], vp[:], bvt[:])
                    nc.gpsimd.dma_start(v_out[ST * st:ST * (st + 1), :], vf[:])
                    nc.vector.tensor_add(
                        vaug[st][:].rearrange("p (h x) -> p h x", h=HL)[:, :, 0:D],
                        vp[:].rearrange("p (h d) -> p h d", h=HL),
                        bvt[:].rearrange("p (h d) -> p h d", h=HL),
                    )

            qk_chunk(0)
            qk_chunk(4)
            v_tiles(range(0, 8))
            qk_chunk(1)
            qk_chunk(5)
            v_tiles(range(8, 16))
            qk_chunk(2)
            qk_chunk(6)
            qk_chunk(3)
            qk_chunk(7)

            # ---- phase 2+3: attention (two heads interleaved, pair-major)
            # with the output projection overlapped per finished q-range ----
            if True:
                for p in range(2):  # q-chunk pairs (0,1) and (2,3)
                    qcs = [2 * p, 2 * p + 1]
                    for hh in range(HL // 2):  # head pairs, interleaved
                        heads = [2 * hh, 2 * hh + 1]
                        apsum = {}
                        for h in heads:
                            for qc in qcs:
                                apsum[h, qc] = aps.tile(
                                    [D + 1, SC], f32, tag="aps",
                                    name=f"ap{h}_{qc}")
                        for j in range(8 * p + 8):
                            for h in heads:
                                kT_h = ktd[h]
                                qT_h = qtd[h]
                                for qc in qcs:
                                    if j > 4 * qc + 3:
                                        continue
                                    q_lo = max(SC * qc, ST * j)
                                    n = SC * (qc + 1) - q_lo
                                    sp = sps.tile([128, SC], f32, tag="sps",
                                                  name=f"sp{h}_{j}_{qc}")
                                    nc.tensor.matmul(
                                        sp[:, 0:n],
                                        kT_h[:][:, ST * j:ST * (j + 1)],
                                        qT_h[:][:, q_lo:q_lo + n],
                                        start=True, stop=True)
                                    et = etp.tile([128, SC], bf16, tag="et",
                                                  name=f"et{h}_{j}_{qc}")
                                    nc.scalar.activation(et[:, 0:n], sp[:, 0:n],
                                                         AF.Exp, scale=0.0625)
                                    if q_lo == ST * j:  # diagonal block
                                        nc.vector.tensor_mul(
                                            et[:, 0:ST], et[:, 0:ST], maskt[:])
                                    nc.tensor.matmul(
                                        apsum[h, qc][:, q_lo - SC * qc:
                                                      q_lo - SC * qc + n],
                                        vaug[j][:, (D + 1) * h:(D + 1) * (h + 1)],
                                        et[:, 0:n],
                                        start=(j == 0), stop=(j == 4 * qc + 3),
                                    )
                        for h in heads:
                            ki = h // 2
                            pr = 64 * (h % 2)
                            for qc in qcs:
                                dc = rp.tile([1, SC], f32, tag="dc",
                                             name=f"dc{h}_{qc}")
                                nc.vector.tensor_copy(dc[:],
                                                      apsum[h, qc][D:D + 1, :])
                                dn = rp.tile([1, SC], f32, tag="dn",
                                             name=f"dn{h}_{qc}")
                                nc.vector.reciprocal_approx_fast(
                                    out=dn[:], in_=dc[:])
                                rb = rp.tile([64, SC], f32, tag="rb",
                                             name=f"rb{h}_{qc}")
                                nc.gpsimd.partition_broadcast(rb[:], dn[:])
                                nc.vector.tensor_mul(
                                    atb[ki][pr:pr + 64, SC * qc:SC * (qc + 1)],
                                    apsum[h, qc][0:D, :], rb[:])
                    # output projection for this pair's q-range (reuses
                    # the score-psum slots; overlaps next pair's attention)
                    for sc in qcs:
                        for oc in range(E // 128):
                            pp = sps.tile([128, SC], f32, tag="sps",
                                          name=f"pp{oc}_{sc}")
                            for cc in range(CL // 128):
                                nc.tensor.matmul(
                                    pp[:],
                                    wpt[cc][:, 128 * oc:128 * (oc + 1)],
                                    atb[cc][:, SC * sc:SC * (sc + 1)],
                                    start=(cc == 0), stop=(cc == CL // 128 - 1),
                                )
                            ot = outp.tile([128, SC], f32, tag="ot",
                                           name=f"ot{oc}_{sc}")
                            nc.vector.tensor_scalar_add(ot[:], pp[:],
                                                        bprojt[:, oc:oc + 1])
                            nc.sync.dma_start(
                                out_t[128 * oc:128 * (oc + 1),
                                      SC * sc:SC * (sc + 1)],
                                ot[:])
            aps.release()
            sps.release()
    nc.finalize()
    return nc


def make_in_maps(hidden_states, w_attn, b_attn, w_proj, b_proj):
    x = np.asarray(hidden_states, dtype=np.float32)
    wa = np.asarray(w_attn, dtype=np.float32)
    ba = np.asarray(b_attn, dtype=np.float32)
    wpj = np.asarray(w_proj, dtype=np.float32)
    bp = np.asarray(b_proj, dtype=np.float32)

    kk, qq = np.meshgrid(np.arange(ST), np.arange(ST), indexing="ij")
    maskT = (qq >= kk).astype(BF16)

    in_maps = []
    for c in range(NCORES):
        b, g = c // G, c % G
        cs = slice(CL * g, CL * (g + 1))
        wq = np.concatenate(
            [wa[:, CL * g:CL * (g + 1)],
             wa[:, E + CL * g:E + CL * (g + 1)],
             wa[:, 2 * E + CL * g:2 * E + CL * (g + 1)]], axis=1)
        bqk = np.zeros((128, 8), np.float32)
        for cc in range(4):
            bqk[:, cc] = ba[CL * g + 128 * cc:CL * g + 128 * (cc + 1)]
            bqk[:, cc + 4] = ba[E + CL * g + 128 * cc:E + CL * g + 128 * (cc + 1)]
        bv = np.tile(ba[2 * E + CL * g:2 * E + CL * (g + 1)][None, :], (128, 1))
        bproj = np.zeros((128, 8), np.float32)
        if g == 0:
            for oc in range(8):
                bproj[:, oc] = bp[128 * oc:128 * (oc + 1)]
        in_maps.append({
            "xT": np.ascontiguousarray(x[b].T).astype(BF16),
            "wqkv": np.ascontiguousarray(wq).astype(BF16),
            "wp": np.ascontiguousarray(wpj[cs, :]).astype(BF16),
            "maskT": maskT,
            "bqk": bqk.astype(np.float32),
            "bv": bv.astype(np.float32),
            "bproj": bproj.astype(np.float32),
        })
    return in_maps


def gather(results):
    out = np.empty((B, S, E), np.float32)
    k_ret = np.empty((B, H, D, S), np.float32)
    v_ret = np.empty((B, H, S, D), np.float32)
    for b in range(B):
        p0 = np.asarray(results[G * b]["out_t"], np.float32)
        p1 = np.asarray(results[G * b + 1]["out_t"], np.float32)
        out[b] = (p0 + p1).T
        for g in range(G):
            r = results[G * b + g]
            k_ret[b, HL * g:HL * (g + 1)] = np.asarray(
                r["kt_out"], np.float32).reshape(HL, D, S)
            v_ret[b, HL * g:HL * (g + 1)] = np.asarray(
                r["v_out"], np.float32).reshape(S, HL, D).transpose(1, 0, 2)
    return out, k_ret, v_ret


def run_on_hw(in_maps, trace=False, **kw):
    from concourse.bass_utils import run_bass_kernel_spmd
    if "nc" not in _cached:
        _cached["nc"] = build_nc()
    return run_bass_kernel_spmd(_cached["nc"], in_maps,
                                core_ids=list(range(NCORES)), trace=trace, **kw)


def kernel(hidden_states, w_attn, b_attn, w_proj, b_proj):
    in_maps = make_in_maps(hidden_states, w_attn, b_attn, w_proj, b_proj)
    res = run_on_hw(in_maps)
    return gather(res.results)
